# revision 34
# baseline (speedup 1.0000x reference)
"""Decoder block kernel for 8 Trainium2 NeuronCores.

Sharding: core = 2*b + h handles batch b, query tokens q with q % 2 == h
(interleaved so the causal-mask block structure is identical on every
core -> one SPMD program; the mask diagonal band differs only in DATA).

All activations live transposed [C, tokens] (C on partitions), so every
linear layer uses the stored [in,out] weights directly as the stationary
operand and no on-device transposes are needed. LayerNorm statistics are
computed with ones-matmuls on the PE (replicated across partitions);
softmax row sums come from a ones-column appended to V. Matmul operands
are float32r (full-rate fp32).
"""
import numpy as np

B, T, C, H, D, FF = 4, 1024, 1024, 16, 64, 4096
NT = C // 128   # 8 partition tiles of the model dim
KT = T // 128   # 8 context-token tiles
FT = FF // 128  # 32
TQ = T // 2     # 512 local query tokens per core

_CACHE = {}


def _build(repeat=1):
    import concourse.bacc as bacc
    import concourse.tile as tile
    from concourse import mybir

    nc = bacc.Bacc(None, target_bir_lowering=False)
    F32 = mybir.dt.float32
    F32R = mybir.dt.float32r

    def din(name, shape, dt=F32R):
        return nc.dram_tensor(name, shape, dt, kind="ExternalInput")

    t = {}
    t["xT"] = din("xT", [C, T])
    t["xTl"] = din("xTl", [C, TQ])
    t["encT"] = din("encT", [C, T])
    for k in ("wq1", "wk1", "wv1", "wo1", "wq2", "wk2", "wv2", "wo2"):
        t[k] = din(k, [C, C])
    t["wf1"] = din("wf1", [C, FF])        # pre-scaled by diag(g2)
    t["csq2"] = din("csq2", [C], F32)     # colsums of g1-scaled Wq2
    t["bq2"] = din("bq2", [C], F32)       # b1 @ Wq2
    t["csf1"] = din("csf1", [FF], F32)    # colsums of g2-scaled Wf1
    t["wf2"] = din("wf2", [FF, C])
    t["tri"] = din("tri", [128, 64])
    t["pad1"] = din("pad1", [T], F32)
    t["pad2"] = din("pad2", [T], F32)
    for k in ("g1", "b1", "g2", "b2", "g3", "b3"):
        t[k] = din(k, [C], F32)
    t["bf1"] = din("bf1", [FF], F32)
    t["bf2"] = din("bf2", [C], F32)
    t["outT"] = nc.dram_tensor("outT", [C, TQ], F32, kind="ExternalOutput")

    with tile.TileContext(nc) as tc:
        for it in range(repeat):
            _emit(nc, tc, t, it)
    nc.compile()
    return nc


def _emit(nc, tc, t, it):
    from contextlib import ExitStack
    import concourse.bass as bass
    from concourse import mybir

    F32 = mybir.dt.float32
    F32R = mybir.dt.float32r
    AF = mybir.ActivationFunctionType
    ALU = mybir.AluOpType

    def vec_ap(dram, n):
        return bass.AP(tensor=dram, offset=0, ap=[[1, 128], [128, n // 128]])

    def w_ap(wdram, cout, ot, a0, na):
        """[128, na, 128] tile: W[128*(a0+a)+p, 128*ot+o]"""
        return bass.AP(tensor=wdram, offset=128 * ot + 128 * a0 * cout,
                       ap=[[cout, 128], [128 * cout, na], [1, 128]])

    with ExitStack() as ctx:
        consts = ctx.enter_context(tc.tile_pool(name=f"con{it}", bufs=1))
        ones128 = consts.tile([128, 128], F32R, tag="o128", name="o128")
        nc.vector.memset(ones128[:].bitcast(F32), 1.0)
        ones1 = consts.tile([1, 128], F32R, tag="o1", name="o1")
        nc.vector.memset(ones1[:].bitcast(F32), 1.0)
        eps_t = consts.tile([128, 1], F32, tag="eps", name="eps")
        nc.vector.memset(eps_t[:], 1e-5)
        tri_sb = consts.tile([128, 64], F32R, tag="tri", name="tri")
        nc.scalar.dma_start(tri_sb[:], t["tri"][:])

        def ldvec(dram, n, tagname):
            s = consts.tile([128, n // 128], F32, tag=tagname, name=tagname)
            nc.scalar.dma_start(s[:], vec_ap(dram, n))
            return s

        g1 = ldvec(t["g1"], C, "g1"); b1 = ldvec(t["b1"], C, "b1")
        g2 = ldvec(t["g2"], C, "g2"); b2 = ldvec(t["b2"], C, "b2")
        g3 = ldvec(t["g3"], C, "g3"); b3 = ldvec(t["b3"], C, "b3")
        bf1s = ldvec(t["bf1"], FF, "bf1")
        csq2s = ldvec(t["csq2"], C, "csq2")
        bq2s = ldvec(t["bq2"], C, "bq2")
        csf1s = ldvec(t["csf1"], FF, "csf1")
        bf2s = ldvec(t["bf2"], C, "bf2")
        pad_sb = {"pad1": ldvec(t["pad1"], T, "pad1"),
                  "pad2": ldvec(t["pad2"], T, "pad2")}

        wpool = ctx.enter_context(tc.tile_pool(name=f"wp{it}", bufs=4))
        ypool = ctx.enter_context(tc.tile_pool(name=f"yp{it}", bufs=1))
        foldp = ctx.enter_context(tc.tile_pool(name=f"fp{it}", bufs=2))

        def fold_epilogue(ps, fold, ot, out_tile, func, bias_sb):
            """out = func(rstd*(ps - m*CS[ot]) + bias) given fold=(rstd, un)
            with un = -m*rstd, CS per-output-channel colsum."""
            rstd, un, cs = fold
            ftile = foldp.tile([128, TQ], F32, tag="ft", name="ft")
            nc.vector.tensor_mul(ftile[:], ps[:], rstd[:])
            nc.vector.scalar_tensor_tensor(
                out=ftile[:], in0=un[:], scalar=cs[:, ot:ot + 1], in1=ftile[:],
                op0=ALU.mult, op1=ALU.add)
            nc.scalar.activation(out_tile[:], ftile[:], func,
                                 bias=bias_sb[:, ot:ot + 1])

        def new_y(dt=F32R):
            return [ypool.tile([128, TQ], dt, tag=f"y{i}", name=f"y{i}")
                    for i in range(NT)]

        def linear_T(wdram, cin, cout, in_tiles, n, epilogue, pp):
            """psum[ot][:, q0:] = sum_ct W[ct,ot].T @ in[ct][:, q0:]"""
            nct = cin // 128
            for ot in range(cout // 128):
                wt = wpool.tile([128, nct, 128], F32R, tag="w", name="w")
                nc.sync.dma_start(wt[:], w_ap(wdram, cout, ot, 0, nct))
                for q0 in range(0, n, 512):
                    ps = pp.tile([128, 512], F32, tag="pp", name="pp")
                    for ct in range(nct):
                        nc.tensor.matmul(ps[:], wt[:, ct, :],
                                         in_tiles[ct][:, q0:q0 + 512],
                                         start=(ct == 0), stop=(ct == nct - 1))
                    epilogue(ot, q0, ps)

        def ln_begin(sctx, tagp, lnps, lntag):
            sqp = sctx.enter_context(tc.tile_pool(name=f"sq{tagp}{it}", bufs=2))
            scr = sctx.enter_context(tc.tile_pool(name=f"ls{tagp}{it}", bufs=1))
            ps1 = lnps.tile([128, TQ], F32, tag=lntag, name="ln")
            ps2 = lnps.tile([128, TQ], F32, tag=lntag, name="ln")
            return {"sqp": sqp, "scr": scr, "ps1": ps1, "ps2": ps2}

        def ln_feed(st, y_tile, ct):
            """Accumulate sum(y) and sum(y^2) for one partition tile."""
            sq = st["sqp"].tile([128, TQ], F32R, tag="sq", name="sq")
            with nc.allow_low_precision(reason="fp32r rounding ok"):
                nc.vector.tensor_mul(sq[:], y_tile[:], y_tile[:])
            nc.tensor.matmul(st["ps1"][:], ones128[:], y_tile[:],
                             start=(ct == 0), stop=(ct == NT - 1),
                             skip_group_check=True)
            nc.tensor.matmul(st["ps2"][:], ones128[:], sq[:],
                             start=(ct == 0), stop=(ct == NT - 1),
                             skip_group_check=True)

        def ln_finish(st, y_in, g, b, out_t, statpool=None):
            """Scalar chain + normalize (overwrites y_in) + affine.
            If statpool is given, m/rstd/un are allocated there and
            (rstd, un) returned for fold_epilogue use by the next stage."""
            scr = st["scr"]
            sp_ = statpool if statpool is not None else scr
            m = sp_.tile([128, TQ], F32, tag="m", name="m")
            nc.vector.tensor_scalar_mul(m[:], st["ps1"][:], 1.0 / C)
            ms = scr.tile([128, TQ], F32, tag="v", name="v")
            nc.vector.tensor_scalar_mul(ms[:], st["ps2"][:], 1.0 / C)
            m2 = scr.tile([128, TQ], F32, tag="v2", name="v2")
            nc.vector.tensor_mul(m2[:], m[:], m[:])
            nc.vector.tensor_sub(ms[:], ms[:], m2[:])
            nc.scalar.activation(ms[:], ms[:], AF.Sqrt, bias=eps_t[:])
            rstd = sp_.tile([128, TQ], F32, tag="r", name="r")
            nc.vector.reciprocal(rstd[:], ms[:])
            un = None
            if statpool is not None:
                un = sp_.tile([128, TQ], F32, tag="un", name="un")
                nc.vector.tensor_mul(un[:], m[:], rstd[:])
                nc.vector.tensor_scalar(un[:], un[:], -1.0, None,
                                        op0=ALU.mult)
            for ct in range(NT):
                eng = nc.vector if ct % 2 == 0 else nc.gpsimd
                d = scr.tile([128, TQ], F32, tag="d", name="d", bufs=2)
                eng.tensor_sub(d[:], y_in[ct][:], m[:])
                eng.tensor_mul(d[:], d[:], rstd[:])
                nc.scalar.activation(out_t[ct][:], d[:], AF.Identity,
                                     bias=b[:, ct:ct + 1], scale=g[:, ct:ct + 1])
            return (rstd, un)

        def kv_stage(src_dram, wk_d, wv_d, wq_d, q_src, padkey, kTt, vtt, qTt,
                     sctx, tagp, q_first, pp, qfold=None):
            """Load transposed source; compute K_T, V (pad-masked, with the
            pad column in slot 64 for the softmax row sums), and Q_T.
            q_first=True when the Q source is already resident (emit Q's
            matmuls before the big src DMAs); False when Q's source is the
            previous stage's LN output (emit K/V first so the PE can run
            while the LN chain finishes)."""
            with ExitStack() as kctx:
                sp = kctx.enter_context(tc.tile_pool(name=f"sr{tagp}{it}",
                                                     bufs=1))

                def emit_q():
                    if qfold is None:
                        linear_T(wq_d, C, C, q_src, TQ,
                                 lambda ot, q0, ps: nc.scalar.copy(
                                     qTt[ot][:], ps[:]), pp)
                    else:
                        linear_T(wq_d, C, C, q_src, TQ,
                                 lambda ot, q0, ps: fold_epilogue(
                                     ps, qfold, ot, qTt[ot], AF.Identity,
                                     bq2s), pp)

                src_t = sp.tile([128, NT, T], F32R, tag="s", name="s")
                src = [src_t[:, i, :] for i in range(NT)]
                if q_first:
                    emit_q()
                src_eng = nc.sync
                for th in range(2):
                    src_eng.dma_start(
                        src_t[:, :, 512 * th:512 * (th + 1)],
                        bass.AP(tensor=src_dram, offset=512 * th,
                                ap=[[T, 128], [128 * T, NT], [1, 512]]))
                linear_T(wk_d, C, C, src, T,
                         lambda ot, q0, ps: nc.scalar.copy(
                             kTt[ot][:, q0:q0 + 512], ps[:]), pp)
                if not q_first:
                    emit_q()
                wvp = kctx.enter_context(tc.tile_pool(name=f"wv{tagp}{it}",
                                                      bufs=2))
                for tt in range(KT):
                    nc.scalar.dma_start(
                        vtt[tt][:, :, 64:65],
                        bass.AP(tensor=t[padkey], offset=128 * tt,
                                ap=[[1, 128], [0, 16], [0, 1]]).bitcast(F32R))
                for half in range(2):
                    wvq = []
                    for cq in range(4):
                        wvt = wvp.tile([128, 2, 512], F32R, tag="wv",
                                       name="wv", bufs=4)
                        nc.sync.dma_start(
                            wvt[:],
                            bass.AP(tensor=wv_d,
                                    offset=512 * half + 256 * cq * C,
                                    ap=[[C, 128], [128 * C, 2], [1, 512]]))
                        wvq.append(wvt)
                    for tt in range(KT):
                        ps = pp.tile([128, 512], F32, tag="pp", name="pp")
                        for ct in range(NT):
                            nc.tensor.matmul(
                                ps[:], src[ct][:, 128 * tt:128 * (tt + 1)],
                                wvq[ct // 2][:, ct % 2, :],
                                start=(ct == 0), stop=(ct == NT - 1))
                        nc.vector.tensor_scalar_mul(
                            vtt[tt][:, 8 * half:8 * (half + 1), 0:64],
                            ps[:].rearrange("p (h d) -> p h d", d=64),
                            pad_sb[padkey][:, tt:tt + 1])

        def attention(qTt, kTt, vtt, wo_d, resid, g, b, out_t, causal,
                      sctx, tagp, scp, avp, rbp, op, statpool=None):
            """Scores/AV in transposed layout; O-projection output (+resid)
            is written back into the qT tiles (dead by then), then LN."""
            with ExitStack() as atx:
                pvp = atx.enter_context(tc.tile_pool(
                    name=f"pv{tagp}{it}", bufs=1))
                ppool = atx.enter_context(tc.tile_pool(
                    name=f"pt{tagp}{it}", bufs=3))
                sbp = atx.enter_context(tc.tile_pool(
                    name=f"sb{tagp}{it}", bufs=1))
                pv = [pvp.tile([128, TQ], F32R, tag=f"pv{i}", name=f"pv{i}")
                      for i in range(NT)]
                for h in range(H):
                    ct, off = h // 2, (h % 2) * 64
                    av = avp.tile([65, 512], F32, tag="av", name="av")
                    pend = None

                    def emit_av(p, c, k):
                        nc.tensor.matmul(av[:, c:512], vtt[k][:, h, 0:65],
                                         p[:, c:512],
                                         start=(k == 0), stop=(k == KT - 1))

                    for kt in range(KT):
                        c0 = min(64 * kt, 256) if causal else 0
                        sp = scp.tile([128, 512], F32, tag="pp", name="pp")
                        Pt = ppool.tile([128, TQ], F32R, tag="P", name="P")
                        nc.tensor.matmul(
                            sp[:, c0:512],
                            kTt[ct][off:off + 64, 128 * kt:128 * (kt + 1)],
                            qTt[ct][off:off + 64, c0:512],
                            start=True, stop=True)
                        nc.scalar.activation(Pt[:, c0:512], sp[:, c0:512],
                                             AF.Exp, scale=0.125)
                        if causal:
                            if 64 * kt > 256:
                                nc.gpsimd.memset(Pt[:, 256:64 * kt].bitcast(F32), 0.0)
                            nc.gpsimd.tensor_mul(
                                Pt[:, 64 * kt:64 * (kt + 1)],
                                Pt[:, 64 * kt:64 * (kt + 1)], tri_sb[:])
                        if pend is not None:
                            emit_av(*pend)
                        pend = (Pt, c0, kt)
                    emit_av(*pend)
                    rinv = sbp.tile([1, 512], F32R, tag="ri", name="ri")
                    with nc.allow_low_precision(reason="fp32r rounding ok"):
                        nc.vector.reciprocal(rinv[:], av[64:65, :])
                    rb_ps = rbp.tile([128, 512], F32, tag="rb", name="rb")
                    nc.tensor.matmul(rb_ps[:], ones1[:], rinv[:],
                                     start=True, stop=True)
                    rb = sbp.tile([64, 512], F32, tag="rs", name="rs")
                    nc.vector.tensor_copy(rb[:], rb_ps[0:64, :])
                    with nc.allow_low_precision(reason="fp32r rounding ok"):
                        nc.vector.tensor_mul(pv[ct][off:off + 64, :],
                                             av[0:64, :], rb[:])
                lnst = ln_begin(atx, tagp, avp, "av")
                for co in range(NT):
                    wt = wpool.tile([128, NT, 128], F32R, tag="w", name="w")
                    nc.sync.dma_start(wt[:], w_ap(wo_d, C, co, 0, NT))
                    ps = op.tile([128, 512], F32, tag="o", name="o")
                    for cc in range(NT):
                        nc.tensor.matmul(ps[:], wt[:, cc, :], pv[cc][:],
                                         start=(cc == 0), stop=(cc == NT - 1))
                    with nc.allow_low_precision(reason="fp32r rounding ok"):
                        nc.vector.tensor_add(qTt[co][:], ps[:], resid[co][:])
                    ln_feed(lnst, qTt[co], co)
                return ln_finish(lnst, qTt, g, b, out_t, statpool)

        # ================= main flow =================
        qpool = ctx.enter_context(tc.tile_pool(name=f"qp{it}", bufs=1))
        statp = ctx.enter_context(tc.tile_pool(name=f"st{it}", bufs=1))

        def new_q(pfx="q"):
            return [qpool.tile([128, TQ], F32R, tag=f"{pfx}{i}",
                               name=f"{pfx}{i}") for i in range(NT)]

        with ExitStack() as actx:
            qkv = actx.enter_context(tc.tile_pool(name=f"qkv{it}", bufs=1))
            pps = actx.enter_context(tc.tile_pool(
                name=f"ps{it}", bufs=3, space="PSUM"))
            avps = actx.enter_context(tc.tile_pool(
                name=f"as{it}", bufs=2, space="PSUM"))
            rbps = actx.enter_context(tc.tile_pool(
                name=f"rs{it}", bufs=1, space="PSUM"))
            ops = actx.enter_context(tc.tile_pool(
                name=f"os{it}", bufs=2, space="PSUM"))

            def new_kv():
                k = [qkv.tile([128, T], F32R, tag=f"k{i}", name=f"k{i}")
                     for i in range(NT)]
                v = [qkv.tile([128, 16, 65], F32R, tag=f"v{i}", name=f"v{i}")
                     for i in range(KT)]
                return k, v

            # ---- self-attention + AddNorm ----
            qT = new_q()
            kTt, vtt = new_kv()
            with ExitStack() as sctx:
                xTl_sb = new_q("x")
                for i in range(NT):
                    nc.sync.dma_start(xTl_sb[i][:],
                                      t["xTl"][128 * i:128 * (i + 1), :])
                kv_stage(t["xT"], t["wk1"], t["wv1"], t["wq1"], xTl_sb,
                         "pad1", kTt, vtt, qT, sctx, "s", True, pps)
                y1 = new_y()
                fold1 = attention(qT, kTt, vtt, t["wo1"], xTl_sb, g1, b1,
                                  y1, True, sctx, "s", pps, avps, rbps, ops,
                                  statpool=statp)

            # ---- cross-attention + AddNorm (fresh tile generations) ----
            qT2 = new_q("x")
            kTt2, vtt2 = new_kv()
            with ExitStack() as cctx:
                kv_stage(t["encT"], t["wk2"], t["wv2"], t["wq2"], qT,
                         "pad2", kTt2, vtt2, qT2, cctx, "c", False, pps,
                         qfold=(fold1[0], fold1[1], csq2s))
                y2 = new_y()
                fold2 = attention(qT2, kTt2, vtt2, t["wo2"], y1, g2, b2, y2,
                                  False, cctx, "c", pps, avps, rbps, ops,
                                  statpool=statp)

        # ---- FFN + AddNorm ----
        with ExitStack() as fctx:
            y3p = fctx.enter_context(tc.tile_pool(name=f"y3{it}", bufs=1))
            y3 = [y3p.tile([128, TQ], F32, tag=f"z{i}", name=f"z{i}")
                  for i in range(NT)]
            lnps3 = fctx.enter_context(tc.tile_pool(
                name=f"l3{it}", bufs=2, space="PSUM"))
            lnst3 = ln_begin(fctx, "f", lnps3, "ln")
            ffold = (fold2[0], fold2[1], csf1s)
            with ExitStack() as mctx:
                hp = mctx.enter_context(tc.tile_pool(name=f"hp{it}", bufs=1))
                w1p = mctx.enter_context(tc.tile_pool(name=f"w1{it}", bufs=4))
                w2p = mctx.enter_context(tc.tile_pool(name=f"w2{it}", bufs=2))
                pp1 = mctx.enter_context(tc.tile_pool(
                    name=f"f1{it}", bufs=3, space="PSUM"))
                pp2 = mctx.enter_context(tc.tile_pool(
                    name=f"f2{it}", bufs=2, space="PSUM"))
                NF = 16
                for fb in range(2):
                    h_sb = [hp.tile([128, TQ], F32R, tag=f"h{i}",
                                    name=f"h{i}") for i in range(NF)]
                    for f in range(NF):
                        fg = fb * NF + f
                        w1t = w1p.tile([128, NT, 128], F32R, tag="w1",
                                       name="w1")
                        nc.sync.dma_start(w1t[:], w_ap(t["wf1"], FF, fg, 0, NT))
                        ps = pp1.tile([128, 512], F32, tag="p1", name="p1")
                        for ct in range(NT):
                            nc.tensor.matmul(ps[:], w1t[:, ct, :], qT2[ct][:],
                                             start=(ct == 0),
                                             stop=(ct == NT - 1))
                        fold_epilogue(ps, ffold, fg, h_sb[f], AF.Relu, bf1s)
                    for co in range(NT):
                        w2t = w2p.tile([128, NF, 128], F32R, tag="w2",
                                       name="w2")
                        nc.sync.dma_start(
                            w2t[:], w_ap(t["wf2"], C, co, fb * NF, NF))
                        ps = pp2.tile([128, 512], F32, tag="p2", name="p2")
                        for f in range(NF):
                            nc.tensor.matmul(ps[:], w2t[:, f, :], h_sb[f][:],
                                             start=(f == 0),
                                             stop=(f == NF - 1))
                        if fb == 0:
                            nc.vector.tensor_copy(y3[co][:], ps[:])
                        else:
                            nc.vector.scalar_tensor_tensor(
                                out=y3[co][:], in0=ps[:],
                                scalar=bf2s[:, co:co + 1], in1=y3[co][:],
                                op0=ALU.add, op1=ALU.add)
                            with nc.allow_low_precision(
                                    reason="fp32r rounding ok"):
                                nc.vector.tensor_add(y2[co][:], y3[co][:],
                                                     y2[co][:])
                            ln_feed(lnst3, y2[co], co)
            ln_finish(lnst3, y2, g3, b3, y3)
            for co in range(NT):
                nc.sync.dma_start(t["outT"][128 * co:128 * (co + 1), :],
                                  y3[co][:])


def _shard(inputs):
    x = np.ascontiguousarray(np.asarray(inputs["x"], dtype=np.float32))
    enc = np.ascontiguousarray(np.asarray(inputs["enc_out"], dtype=np.float32))
    tpad = np.asarray(inputs["tgt_pad_mask"]).astype(np.float32)
    spad = np.asarray(inputs["src_pad_mask"]).astype(np.float32)
    ws = {k: np.ascontiguousarray(np.asarray(inputs[k], dtype=np.float32))
          for k in ("Wq1", "Wk1", "Wv1", "Wo1", "Wq2", "Wk2", "Wv2", "Wo2",
                    "Wf1", "Wf2")}
    lnv = {k: np.ascontiguousarray(np.asarray(inputs[k], dtype=np.float32))
           for k in ("ln1_g", "ln1_b", "ln2_g", "ln2_b", "ln3_g", "ln3_b",
                     "bf1", "bf2")}
    # LN1 affine folded through Wq2; LN2 affine folded through Wf1.
    wq2f = np.ascontiguousarray(lnv["ln1_g"][:, None] * ws["Wq2"])
    csq2 = np.ascontiguousarray(wq2f.sum(axis=0))
    bq2 = np.ascontiguousarray(lnv["ln1_b"] @ ws["Wq2"])
    wf1f = np.ascontiguousarray(lnv["ln2_g"][:, None] * ws["Wf1"])
    csf1 = np.ascontiguousarray(wf1f.sum(axis=0))
    bf1f = np.ascontiguousarray(lnv["bf1"] + lnv["ln2_b"] @ ws["Wf1"])
    in_maps = []
    for b in range(B):
        xTb = np.ascontiguousarray(x[b].T)
        eTb = np.ascontiguousarray(enc[b].T)
        p1 = np.ascontiguousarray(1.0 - tpad[b])
        p2 = np.ascontiguousarray(1.0 - spad[b])
        for h in range(2):
            xTlb = np.ascontiguousarray(x[b, h::2, :].T)
            trih = (np.arange(128)[:, None] <= 2 * np.arange(64)[None, :] + h
                    ).astype(np.float32)
            in_maps.append({
                "xT": xTb, "xTl": xTlb, "encT": eTb,
                "wq1": ws["Wq1"], "wk1": ws["Wk1"], "wv1": ws["Wv1"],
                "wo1": ws["Wo1"], "wq2": wq2f, "wk2": ws["Wk2"],
                "wv2": ws["Wv2"], "wo2": ws["Wo2"],
                "wf1": wf1f, "wf2": ws["Wf2"],
                "csq2": csq2, "bq2": bq2, "csf1": csf1,
                "tri": np.ascontiguousarray(trih),
                "pad1": p1, "pad2": p2,
                "g1": lnv["ln1_g"], "b1": lnv["ln1_b"],
                "g2": lnv["ln2_g"], "b2": lnv["ln2_b"],
                "g3": lnv["ln3_g"], "b3": lnv["ln3_b"],
                "bf1": bf1f, "bf2": lnv["bf2"],
            })
    return in_maps


def _get_nc(repeat=1):
    if repeat not in _CACHE:
        _CACHE[repeat] = _build(repeat)
    return _CACHE[repeat]


def kernel(**inputs):
    from concourse.bass_utils import run_bass_kernel_spmd
    nc = _get_nc()
    in_maps = _shard(inputs)
    res = run_bass_kernel_spmd(nc, in_maps, core_ids=list(range(8)))
    out = np.empty((B, T, C), np.float32)
    for core in range(8):
        b, h = core // 2, core % 2
        out[b, h::2, :] = res.results[core]["outT"].T
    return out


# revision 37
# speedup vs baseline: 1.0081x; 1.0081x over previous
"""Decoder block kernel for 8 Trainium2 NeuronCores.

Sharding: core = 2*b + h handles batch b, query tokens q with q % 2 == h
(interleaved so the causal-mask block structure is identical on every
core -> one SPMD program; the mask diagonal band differs only in DATA).

All activations live transposed [C, tokens] (C on partitions), so every
linear layer uses the stored [in,out] weights directly as the stationary
operand and no on-device transposes are needed. LayerNorm statistics are
computed with ones-matmuls on the PE (replicated across partitions);
softmax row sums come from a ones-column appended to V. Matmul operands
are float32r (full-rate fp32).
"""
import numpy as np

B, T, C, H, D, FF = 4, 1024, 1024, 16, 64, 4096
NT = C // 128   # 8 partition tiles of the model dim
KT = T // 128   # 8 context-token tiles
FT = FF // 128  # 32
TQ = T // 2     # 512 local query tokens per core

_CACHE = {}


def _build(repeat=1):
    import concourse.bacc as bacc
    import concourse.tile as tile
    from concourse import mybir

    nc = bacc.Bacc(None, target_bir_lowering=False)
    F32 = mybir.dt.float32
    F32R = mybir.dt.float32r

    def din(name, shape, dt=F32R):
        return nc.dram_tensor(name, shape, dt, kind="ExternalInput")

    t = {}
    t["xT"] = din("xT", [C, T])
    t["xTl"] = din("xTl", [C, TQ])
    t["encT"] = din("encT", [C, T])
    for k in ("wq1", "wk1", "wv1", "wo1", "wq2", "wk2", "wv2", "wo2"):
        t[k] = din(k, [C, C])
    t["wf1"] = din("wf1", [C, FF])        # pre-scaled by diag(g2)
    t["csq2"] = din("csq2", [C], F32)     # colsums of g1-scaled Wq2
    t["bq2"] = din("bq2", [C], F32)       # b1 @ Wq2
    t["csf1"] = din("csf1", [FF], F32)    # colsums of g2-scaled Wf1
    t["wf2"] = din("wf2", [FF, C])
    t["tri"] = din("tri", [128, 64])
    t["pad1"] = din("pad1", [T], F32)
    t["pad2"] = din("pad2", [T], F32)
    for k in ("g1", "b1", "g2", "b2", "g3", "b3"):
        t[k] = din(k, [C], F32)
    t["bf1"] = din("bf1", [FF], F32)
    t["bf2"] = din("bf2", [C], F32)
    t["outT"] = nc.dram_tensor("outT", [C, TQ], F32, kind="ExternalOutput")

    with tile.TileContext(nc) as tc:
        for it in range(repeat):
            _emit(nc, tc, t, it)
    nc.compile()
    return nc


def _emit(nc, tc, t, it):
    from contextlib import ExitStack
    import concourse.bass as bass
    from concourse import mybir
    from concourse.tile import add_dep_helper

    F32 = mybir.dt.float32
    F32R = mybir.dt.float32r
    AF = mybir.ActivationFunctionType
    ALU = mybir.AluOpType

    def vec_ap(dram, n):
        return bass.AP(tensor=dram, offset=0, ap=[[1, 128], [128, n // 128]])

    def w_ap(wdram, cout, ot, a0, na):
        """[128, na, 128] tile: W[128*(a0+a)+p, 128*ot+o]"""
        return bass.AP(tensor=wdram, offset=128 * ot + 128 * a0 * cout,
                       ap=[[cout, 128], [128 * cout, na], [1, 128]])

    with ExitStack() as ctx:
        consts = ctx.enter_context(tc.tile_pool(name=f"con{it}", bufs=1))
        ones128 = consts.tile([128, 128], F32R, tag="o128", name="o128")
        nc.vector.memset(ones128[:].bitcast(F32), 1.0)
        ones1 = consts.tile([1, 128], F32R, tag="o1", name="o1")
        nc.vector.memset(ones1[:].bitcast(F32), 1.0)
        eps_t = consts.tile([128, 1], F32, tag="eps", name="eps")
        nc.vector.memset(eps_t[:], 1e-5)
        tri_sb = consts.tile([128, 64], F32R, tag="tri", name="tri")
        nc.scalar.dma_start(tri_sb[:], t["tri"][:])

        def ldvec(dram, n, tagname):
            s = consts.tile([128, n // 128], F32, tag=tagname, name=tagname)
            nc.scalar.dma_start(s[:], vec_ap(dram, n))
            return s

        g1 = ldvec(t["g1"], C, "g1"); b1 = ldvec(t["b1"], C, "b1")
        g2 = ldvec(t["g2"], C, "g2"); b2 = ldvec(t["b2"], C, "b2")
        g3 = ldvec(t["g3"], C, "g3"); b3 = ldvec(t["b3"], C, "b3")
        bf1s = ldvec(t["bf1"], FF, "bf1")
        csq2s = ldvec(t["csq2"], C, "csq2")
        bq2s = ldvec(t["bq2"], C, "bq2")
        csf1s = ldvec(t["csf1"], FF, "csf1")
        bf2s = ldvec(t["bf2"], C, "bf2")
        pad_sb = {"pad1": ldvec(t["pad1"], T, "pad1"),
                  "pad2": ldvec(t["pad2"], T, "pad2")}

        wpool = ctx.enter_context(tc.tile_pool(name=f"wp{it}", bufs=4))
        ypool = ctx.enter_context(tc.tile_pool(name=f"yp{it}", bufs=1))
        foldp = ctx.enter_context(tc.tile_pool(name=f"fp{it}", bufs=2))

        def fold_epilogue(ps, fold, ot, out_tile, func, bias_sb):
            """out = func(rstd*(ps - m*CS[ot]) + bias) given fold=(rstd, un)
            with un = -m*rstd, CS per-output-channel colsum."""
            rstd, un, cs = fold
            ftile = foldp.tile([128, TQ], F32, tag="ft", name="ft")
            nc.vector.tensor_mul(ftile[:], ps[:], rstd[:])
            nc.vector.scalar_tensor_tensor(
                out=ftile[:], in0=un[:], scalar=cs[:, ot:ot + 1], in1=ftile[:],
                op0=ALU.mult, op1=ALU.add)
            nc.scalar.activation(out_tile[:], ftile[:], func,
                                 bias=bias_sb[:, ot:ot + 1])

        def new_y(dt=F32R):
            return [ypool.tile([128, TQ], dt, tag=f"y{i}", name=f"y{i}")
                    for i in range(NT)]

        def linear_T(wdram, cin, cout, in_tiles, n, epilogue, pp):
            """psum[ot][:, q0:] = sum_ct W[ct,ot].T @ in[ct][:, q0:]"""
            nct = cin // 128
            for ot in range(cout // 128):
                wt = wpool.tile([128, nct, 128], F32R, tag="w", name="w")
                nc.sync.dma_start(wt[:], w_ap(wdram, cout, ot, 0, nct))
                for q0 in range(0, n, 512):
                    ps = pp.tile([128, 512], F32, tag="pp", name="pp")
                    for ct in range(nct):
                        nc.tensor.matmul(ps[:], wt[:, ct, :],
                                         in_tiles[ct][:, q0:q0 + 512],
                                         start=(ct == 0), stop=(ct == nct - 1))
                    epilogue(ot, q0, ps)

        def ln_begin(sctx, tagp, lnps, lntag):
            sqp = sctx.enter_context(tc.tile_pool(name=f"sq{tagp}{it}", bufs=2))
            scr = sctx.enter_context(tc.tile_pool(name=f"ls{tagp}{it}", bufs=1))
            ps1 = lnps.tile([128, TQ], F32, tag=lntag, name="ln")
            ps2 = lnps.tile([128, TQ], F32, tag=lntag, name="ln")
            return {"sqp": sqp, "scr": scr, "ps1": ps1, "ps2": ps2}

        def ln_feed(st, y_tile, ct):
            """Accumulate sum(y) and sum(y^2) for one partition tile."""
            sq = st["sqp"].tile([128, TQ], F32R, tag="sq", name="sq")
            with nc.allow_low_precision(reason="fp32r rounding ok"):
                nc.vector.tensor_mul(sq[:], y_tile[:], y_tile[:])
            nc.tensor.matmul(st["ps1"][:], ones128[:], y_tile[:],
                             start=(ct == 0), stop=(ct == NT - 1),
                             skip_group_check=True)
            nc.tensor.matmul(st["ps2"][:], ones128[:], sq[:],
                             start=(ct == 0), stop=(ct == NT - 1),
                             skip_group_check=True)

        def ln_finish(st, y_in, g, b, out_t, statpool=None):
            """Scalar chain + normalize (overwrites y_in) + affine.
            If statpool is given, m/rstd/un are allocated there and
            (rstd, un) returned for fold_epilogue use by the next stage."""
            scr = st["scr"]
            sp_ = statpool if statpool is not None else scr
            m = sp_.tile([128, TQ], F32, tag="m", name="m")
            nc.vector.tensor_scalar_mul(m[:], st["ps1"][:], 1.0 / C)
            ms = scr.tile([128, TQ], F32, tag="v", name="v")
            nc.vector.tensor_scalar_mul(ms[:], st["ps2"][:], 1.0 / C)
            m2 = scr.tile([128, TQ], F32, tag="v2", name="v2")
            nc.vector.tensor_mul(m2[:], m[:], m[:])
            nc.vector.tensor_sub(ms[:], ms[:], m2[:])
            nc.scalar.activation(ms[:], ms[:], AF.Sqrt, bias=eps_t[:])
            rstd = sp_.tile([128, TQ], F32, tag="r", name="r")
            nc.vector.reciprocal(rstd[:], ms[:])
            un = None
            if statpool is not None:
                un = sp_.tile([128, TQ], F32, tag="un", name="un")
                nc.vector.tensor_mul(un[:], m[:], rstd[:])
                nc.vector.tensor_scalar(un[:], un[:], -1.0, None,
                                        op0=ALU.mult)
            for ct in range(NT):
                eng = nc.vector if ct % 2 == 0 else nc.gpsimd
                d = scr.tile([128, TQ], F32, tag="d", name="d", bufs=2)
                eng.tensor_sub(d[:], y_in[ct][:], m[:])
                eng.tensor_mul(d[:], d[:], rstd[:])
                nc.scalar.activation(out_t[ct][:], d[:], AF.Identity,
                                     bias=b[:, ct:ct + 1], scale=g[:, ct:ct + 1])
            return (rstd, un)

        def kv_stage(src_dram, wk_d, wv_d, wq_d, q_src, padkey, kTt, vtt, qTt,
                     sctx, tagp, q_first, pp, qfold=None, src_after=None):
            """Load transposed source; compute K_T, V (pad-masked, with the
            pad column in slot 64 for the softmax row sums), and Q_T.
            q_first=True when the Q source is already resident (emit Q's
            matmuls before the big src DMAs); False when Q's source is the
            previous stage's LN output (emit K/V first so the PE can run
            while the LN chain finishes)."""
            with ExitStack() as kctx:
                sp = kctx.enter_context(tc.tile_pool(name=f"sr{tagp}{it}",
                                                     bufs=1))

                def emit_q():
                    if qfold is None:
                        linear_T(wq_d, C, C, q_src, TQ,
                                 lambda ot, q0, ps: nc.scalar.copy(
                                     qTt[ot][:], ps[:]), pp)
                    else:
                        linear_T(wq_d, C, C, q_src, TQ,
                                 lambda ot, q0, ps: fold_epilogue(
                                     ps, qfold, ot, qTt[ot], AF.Identity,
                                     bq2s), pp)

                src_t = sp.tile([128, NT, T], F32R, tag="s", name="s")
                src = [src_t[:, i, :] for i in range(NT)]
                if q_first:
                    emit_q()
                src_eng = nc.gpsimd if q_first else nc.sync
                for th in range(2):
                    di = src_eng.dma_start(
                        src_t[:, :, 512 * th:512 * (th + 1)],
                        bass.AP(tensor=src_dram, offset=512 * th,
                                ap=[[T, 128], [128 * T, NT], [1, 512]]))
                    if th == 0 and src_after is not None:
                        add_dep_helper(di.ins, src_after.ins,
                                       reason="xT transfer after xTl")
                linear_T(wk_d, C, C, src, T,
                         lambda ot, q0, ps: nc.scalar.copy(
                             kTt[ot][:, q0:q0 + 512], ps[:]), pp)
                if not q_first:
                    emit_q()
                wvp = kctx.enter_context(tc.tile_pool(name=f"wv{tagp}{it}",
                                                      bufs=2))
                for tt in range(KT):
                    nc.scalar.dma_start(
                        vtt[tt][:, :, 64:65],
                        bass.AP(tensor=t[padkey], offset=128 * tt,
                                ap=[[1, 128], [0, 16], [0, 1]]).bitcast(F32R))
                for half in range(2):
                    wvq = []
                    for cq in range(4):
                        wvt = wvp.tile([128, 2, 512], F32R, tag="wv",
                                       name="wv", bufs=4)
                        nc.sync.dma_start(
                            wvt[:],
                            bass.AP(tensor=wv_d,
                                    offset=512 * half + 256 * cq * C,
                                    ap=[[C, 128], [128 * C, 2], [1, 512]]))
                        wvq.append(wvt)
                    for tt in range(KT):
                        ps = pp.tile([128, 512], F32, tag="pp", name="pp")
                        for ct in range(NT):
                            nc.tensor.matmul(
                                ps[:], src[ct][:, 128 * tt:128 * (tt + 1)],
                                wvq[ct // 2][:, ct % 2, :],
                                start=(ct == 0), stop=(ct == NT - 1))
                        nc.vector.tensor_scalar_mul(
                            vtt[tt][:, 8 * half:8 * (half + 1), 0:64],
                            ps[:].rearrange("p (h d) -> p h d", d=64),
                            pad_sb[padkey][:, tt:tt + 1])

        def attention(qTt, kTt, vtt, wo_d, resid, g, b, out_t, causal,
                      sctx, tagp, scp, avp, rbp, op, statpool=None):
            """Scores/AV in transposed layout; O-projection output (+resid)
            is written back into the qT tiles (dead by then), then LN."""
            with ExitStack() as atx:
                pvp = atx.enter_context(tc.tile_pool(
                    name=f"pv{tagp}{it}", bufs=1))
                ppool = atx.enter_context(tc.tile_pool(
                    name=f"pt{tagp}{it}", bufs=3))
                sbp = atx.enter_context(tc.tile_pool(
                    name=f"sb{tagp}{it}", bufs=1))
                pv = [pvp.tile([128, TQ], F32R, tag=f"pv{i}", name=f"pv{i}")
                      for i in range(NT)]
                for h in range(H):
                    ct, off = h // 2, (h % 2) * 64
                    av = avp.tile([65, 512], F32, tag="av", name="av")
                    pend = None

                    def emit_av(p, c, k):
                        nc.tensor.matmul(av[:, c:512], vtt[k][:, h, 0:65],
                                         p[:, c:512],
                                         start=(k == 0), stop=(k == KT - 1))

                    for kt in range(KT):
                        c0 = min(64 * kt, 256) if causal else 0
                        sp = scp.tile([128, 512], F32, tag="pp", name="pp")
                        Pt = ppool.tile([128, TQ], F32R, tag="P", name="P")
                        nc.tensor.matmul(
                            sp[:, c0:512],
                            kTt[ct][off:off + 64, 128 * kt:128 * (kt + 1)],
                            qTt[ct][off:off + 64, c0:512],
                            start=True, stop=True)
                        nc.scalar.activation(Pt[:, c0:512], sp[:, c0:512],
                                             AF.Exp, scale=0.125)
                        if causal:
                            if 64 * kt > 256:
                                nc.gpsimd.memset(Pt[:, 256:64 * kt].bitcast(F32), 0.0)
                            nc.gpsimd.tensor_mul(
                                Pt[:, 64 * kt:64 * (kt + 1)],
                                Pt[:, 64 * kt:64 * (kt + 1)], tri_sb[:])
                        if pend is not None:
                            emit_av(*pend)
                        pend = (Pt, c0, kt)
                    emit_av(*pend)
                    rinv = sbp.tile([1, 512], F32R, tag="ri", name="ri")
                    with nc.allow_low_precision(reason="fp32r rounding ok"):
                        nc.vector.reciprocal(rinv[:], av[64:65, :])
                    rb_ps = rbp.tile([128, 512], F32, tag="rb", name="rb")
                    nc.tensor.matmul(rb_ps[:], ones1[:], rinv[:],
                                     start=True, stop=True)
                    rb = sbp.tile([64, 512], F32, tag="rs", name="rs")
                    nc.vector.tensor_copy(rb[:], rb_ps[0:64, :])
                    with nc.allow_low_precision(reason="fp32r rounding ok"):
                        nc.vector.tensor_mul(pv[ct][off:off + 64, :],
                                             av[0:64, :], rb[:])
                lnst = ln_begin(atx, tagp, avp, "av")
                for co in range(NT):
                    wt = wpool.tile([128, NT, 128], F32R, tag="w", name="w")
                    nc.sync.dma_start(wt[:], w_ap(wo_d, C, co, 0, NT))
                    ps = op.tile([128, 512], F32, tag="o", name="o")
                    for cc in range(NT):
                        nc.tensor.matmul(ps[:], wt[:, cc, :], pv[cc][:],
                                         start=(cc == 0), stop=(cc == NT - 1))
                    with nc.allow_low_precision(reason="fp32r rounding ok"):
                        nc.vector.tensor_add(qTt[co][:], ps[:], resid[co][:])
                    ln_feed(lnst, qTt[co], co)
                return ln_finish(lnst, qTt, g, b, out_t, statpool)

        # ================= main flow =================
        qpool = ctx.enter_context(tc.tile_pool(name=f"qp{it}", bufs=1))
        statp = ctx.enter_context(tc.tile_pool(name=f"st{it}", bufs=1))

        def new_q(pfx="q"):
            return [qpool.tile([128, TQ], F32R, tag=f"{pfx}{i}",
                               name=f"{pfx}{i}") for i in range(NT)]

        with ExitStack() as actx:
            qkv = actx.enter_context(tc.tile_pool(name=f"qkv{it}", bufs=1))
            pps = actx.enter_context(tc.tile_pool(
                name=f"ps{it}", bufs=3, space="PSUM"))
            avps = actx.enter_context(tc.tile_pool(
                name=f"as{it}", bufs=2, space="PSUM"))
            rbps = actx.enter_context(tc.tile_pool(
                name=f"rs{it}", bufs=1, space="PSUM"))
            ops = actx.enter_context(tc.tile_pool(
                name=f"os{it}", bufs=2, space="PSUM"))

            def new_kv():
                k = [qkv.tile([128, T], F32R, tag=f"k{i}", name=f"k{i}")
                     for i in range(NT)]
                v = [qkv.tile([128, 16, 65], F32R, tag=f"v{i}", name=f"v{i}")
                     for i in range(KT)]
                return k, v

            # ---- self-attention + AddNorm ----
            qT = new_q()
            kTt, vtt = new_kv()
            with ExitStack() as sctx:
                xTl_sb = new_q("x")
                last_xtl = None
                for i in range(NT):
                    last_xtl = nc.sync.dma_start(
                        xTl_sb[i][:], t["xTl"][128 * i:128 * (i + 1), :])
                kv_stage(t["xT"], t["wk1"], t["wv1"], t["wq1"], xTl_sb,
                         "pad1", kTt, vtt, qT, sctx, "s", True, pps,
                         src_after=last_xtl)
                y1 = new_y()
                fold1 = attention(qT, kTt, vtt, t["wo1"], xTl_sb, g1, b1,
                                  y1, True, sctx, "s", pps, avps, rbps, ops,
                                  statpool=statp)

            # ---- cross-attention + AddNorm (fresh tile generations) ----
            qT2 = new_q("x")
            kTt2, vtt2 = new_kv()
            with ExitStack() as cctx:
                kv_stage(t["encT"], t["wk2"], t["wv2"], t["wq2"], qT,
                         "pad2", kTt2, vtt2, qT2, cctx, "c", False, pps,
                         qfold=(fold1[0], fold1[1], csq2s))
                y2 = new_y()
                fold2 = attention(qT2, kTt2, vtt2, t["wo2"], y1, g2, b2, y2,
                                  False, cctx, "c", pps, avps, rbps, ops,
                                  statpool=statp)

        # ---- FFN + AddNorm ----
        with ExitStack() as fctx:
            y3p = fctx.enter_context(tc.tile_pool(name=f"y3{it}", bufs=1))
            y3 = [y3p.tile([128, TQ], F32, tag=f"z{i}", name=f"z{i}")
                  for i in range(NT)]
            lnps3 = fctx.enter_context(tc.tile_pool(
                name=f"l3{it}", bufs=2, space="PSUM"))
            lnst3 = ln_begin(fctx, "f", lnps3, "ln")
            ffold = (fold2[0], fold2[1], csf1s)
            with ExitStack() as mctx:
                hp = mctx.enter_context(tc.tile_pool(name=f"hp{it}", bufs=1))
                w1p = mctx.enter_context(tc.tile_pool(name=f"w1{it}", bufs=4))
                w2p = mctx.enter_context(tc.tile_pool(name=f"w2{it}", bufs=2))
                pp1 = mctx.enter_context(tc.tile_pool(
                    name=f"f1{it}", bufs=3, space="PSUM"))
                pp2 = mctx.enter_context(tc.tile_pool(
                    name=f"f2{it}", bufs=2, space="PSUM"))
                NF = 16
                for fb in range(2):
                    h_sb = [hp.tile([128, TQ], F32R, tag=f"h{i}",
                                    name=f"h{i}") for i in range(NF)]
                    for f in range(NF):
                        fg = fb * NF + f
                        w1t = w1p.tile([128, NT, 128], F32R, tag="w1",
                                       name="w1")
                        nc.sync.dma_start(w1t[:], w_ap(t["wf1"], FF, fg, 0, NT))
                        ps = pp1.tile([128, 512], F32, tag="p1", name="p1")
                        for ct in range(NT):
                            nc.tensor.matmul(ps[:], w1t[:, ct, :], qT2[ct][:],
                                             start=(ct == 0),
                                             stop=(ct == NT - 1))
                        fold_epilogue(ps, ffold, fg, h_sb[f], AF.Relu, bf1s)
                    for co in range(NT):
                        w2t = w2p.tile([128, NF, 128], F32R, tag="w2",
                                       name="w2")
                        nc.sync.dma_start(
                            w2t[:], w_ap(t["wf2"], C, co, fb * NF, NF))
                        ps = pp2.tile([128, 512], F32, tag="p2", name="p2")
                        for f in range(NF):
                            nc.tensor.matmul(ps[:], w2t[:, f, :], h_sb[f][:],
                                             start=(f == 0),
                                             stop=(f == NF - 1))
                        if fb == 0:
                            nc.vector.tensor_copy(y3[co][:], ps[:])
                        else:
                            nc.vector.scalar_tensor_tensor(
                                out=y3[co][:], in0=ps[:],
                                scalar=bf2s[:, co:co + 1], in1=y3[co][:],
                                op0=ALU.add, op1=ALU.add)
                            with nc.allow_low_precision(
                                    reason="fp32r rounding ok"):
                                nc.vector.tensor_add(y2[co][:], y3[co][:],
                                                     y2[co][:])
                            ln_feed(lnst3, y2[co], co)
            ln_finish(lnst3, y2, g3, b3, y3)
            for co in range(NT):
                nc.sync.dma_start(t["outT"][128 * co:128 * (co + 1), :],
                                  y3[co][:])


def _shard(inputs):
    x = np.ascontiguousarray(np.asarray(inputs["x"], dtype=np.float32))
    enc = np.ascontiguousarray(np.asarray(inputs["enc_out"], dtype=np.float32))
    tpad = np.asarray(inputs["tgt_pad_mask"]).astype(np.float32)
    spad = np.asarray(inputs["src_pad_mask"]).astype(np.float32)
    ws = {k: np.ascontiguousarray(np.asarray(inputs[k], dtype=np.float32))
          for k in ("Wq1", "Wk1", "Wv1", "Wo1", "Wq2", "Wk2", "Wv2", "Wo2",
                    "Wf1", "Wf2")}
    lnv = {k: np.ascontiguousarray(np.asarray(inputs[k], dtype=np.float32))
           for k in ("ln1_g", "ln1_b", "ln2_g", "ln2_b", "ln3_g", "ln3_b",
                     "bf1", "bf2")}
    # LN1 affine folded through Wq2; LN2 affine folded through Wf1.
    wq2f = np.ascontiguousarray(lnv["ln1_g"][:, None] * ws["Wq2"])
    csq2 = np.ascontiguousarray(wq2f.sum(axis=0))
    bq2 = np.ascontiguousarray(lnv["ln1_b"] @ ws["Wq2"])
    wf1f = np.ascontiguousarray(lnv["ln2_g"][:, None] * ws["Wf1"])
    csf1 = np.ascontiguousarray(wf1f.sum(axis=0))
    bf1f = np.ascontiguousarray(lnv["bf1"] + lnv["ln2_b"] @ ws["Wf1"])
    in_maps = []
    for b in range(B):
        xTb = np.ascontiguousarray(x[b].T)
        eTb = np.ascontiguousarray(enc[b].T)
        p1 = np.ascontiguousarray(1.0 - tpad[b])
        p2 = np.ascontiguousarray(1.0 - spad[b])
        for h in range(2):
            xTlb = np.ascontiguousarray(x[b, h::2, :].T)
            trih = (np.arange(128)[:, None] <= 2 * np.arange(64)[None, :] + h
                    ).astype(np.float32)
            in_maps.append({
                "xT": xTb, "xTl": xTlb, "encT": eTb,
                "wq1": ws["Wq1"], "wk1": ws["Wk1"], "wv1": ws["Wv1"],
                "wo1": ws["Wo1"], "wq2": wq2f, "wk2": ws["Wk2"],
                "wv2": ws["Wv2"], "wo2": ws["Wo2"],
                "wf1": wf1f, "wf2": ws["Wf2"],
                "csq2": csq2, "bq2": bq2, "csf1": csf1,
                "tri": np.ascontiguousarray(trih),
                "pad1": p1, "pad2": p2,
                "g1": lnv["ln1_g"], "b1": lnv["ln1_b"],
                "g2": lnv["ln2_g"], "b2": lnv["ln2_b"],
                "g3": lnv["ln3_g"], "b3": lnv["ln3_b"],
                "bf1": bf1f, "bf2": lnv["bf2"],
            })
    return in_maps


def _get_nc(repeat=1):
    if repeat not in _CACHE:
        _CACHE[repeat] = _build(repeat)
    return _CACHE[repeat]


def kernel(**inputs):
    from concourse.bass_utils import run_bass_kernel_spmd
    nc = _get_nc()
    in_maps = _shard(inputs)
    res = run_bass_kernel_spmd(nc, in_maps, core_ids=list(range(8)))
    out = np.empty((B, T, C), np.float32)
    for core in range(8):
        b, h = core // 2, core % 2
        out[b, h::2, :] = res.results[core]["outT"].T
    return out
